# revision 1
# baseline (speedup 1.0000x reference)
"""BiLSTM-CRF NLL loss kernel for 8 Trainium2 NeuronCores.

Data-parallel over batch (128 samples/core). The partition function runs
as a linear-domain recurrence
    p_t = (M^T p_{t-1}) * exp(feats_t - dc_t)
with a host-computed per-step scalar normalizer schedule dc_t, in a
[32 tags x 128 samples] layout: PE does 2 small bf16 matmuls per step
(stationary 32x32 exp(transitions)), DVE one PSUM*SBUF multiply per step
per 64-sample chain; the two chains are phase-staggered so PE/DVE
round-trip latency overlaps. p_t history streams to DRAM in
partition-contiguous blocks; the host does the length-indexed readout
log(sum_j e^{trans[j,STOP]} p_t*[j]) + C_t in fp64.

The host pre-computes ef = exp(feats - dc) in bf16 and uploads it in the
on-chip layout (descriptor-friendly; every DMA is one fat contiguous run
per partition). Gold-score emissions are gathered on GPSIMD via
indirect_copy (indices shared per 16-partition group; a host-built mask
kills off-target rows) and reduced with partition_all_reduce, so the gold
path never touches PE/DVE; transition terms of the gold score are pure
(tags, transitions) index math on the host in fp64.
"""
import numpy as np
import ml_dtypes

B, L, T = 1024, 512, 32
START, STOP = 30, 31
NCORES = 8
BS = B // NCORES          # 128 samples per core
NBLK = 4                  # l-blocks
LB = L // NBLK            # 128 timesteps per block
CH = BS // 2              # 64 samples per chain

_PROG = None

TRACE = False
LAST_EXEC_NS = None


def _build_program():
    import concourse.bacc as bacc
    import concourse.mybir as mybir
    import concourse.tile as tile
    import concourse.bass_isa as bass_isa
    from concourse.tile_rust import add_dep_helper

    F32 = mybir.dt.float32
    BF16 = mybir.dt.bfloat16
    U16 = mybir.dt.uint16
    MULT = mybir.AluOpType.mult

    nc = bacc.Bacc("TRN2", target_bir_lowering=False, debug=False)

    # aef[32k+j, b*128+l_in] = exp(feats[b, 128k+l_in, j] - dc[128k+l_in]) bf16
    aef = nc.dram_tensor("aef", [128, BS * LB], BF16, kind="ExternalInput").ap()
    # ast[k, l_in, b*32+j] = feats[b, 128k+l_in, j] bf16 (gold-emission staging)
    ast = nc.dram_tensor("ast", [NBLK, LB, BS * T], BF16, kind="ExternalInput").ap()
    m32 = nc.dram_tensor("m32", [32, 32], BF16, kind="ExternalInput").ap()
    estart = nc.dram_tensor("estart", [32, 1], F32, kind="ExternalInput").ap()
    eidx = nc.dram_tensor("eidx", [128, NBLK * 128], U16, kind="ExternalInput").ap()
    emask = nc.dram_tensor("emask", [128, NBLK * 2048], BF16, kind="ExternalInput").ap()
    # hist[k, j, t_in*128 + b] = p_t[j, b] at t = 128k + t_in
    hist = nc.dram_tensor("hist", [NBLK, 32, LB * BS], BF16, kind="ExternalOutput").ap()
    emito = nc.dram_tensor("emito", [NBLK, 2048], F32, kind="ExternalOutput").ap()

    with tile.TileContext(nc) as tc:
        with (
            tc.tile_pool(name="consts", bufs=1) as consts,
            tc.tile_pool(name="efpool", bufs=1) as efpool,
            tc.tile_pool(name="stage", bufs=2) as stage,
            tc.tile_pool(name="goldp", bufs=2) as goldp,
            tc.tile_pool(name="ringp", bufs=2) as ringp,
            tc.tile_pool(name="upool", bufs=2, space="PSUM") as upool,
        ):
            m32_sb = consts.tile([32, 32], BF16)
            estart_sb = consts.tile([32, 1], F32)
            eidx_sb = consts.tile([128, NBLK * 128], U16)
            emask_sb = consts.tile([128, NBLK * 2048], BF16)
            nc.sync.dma_start(m32_sb[:], m32[:])
            nc.sync.dma_start(estart_sb[:], estart[:])
            nc.sync.dma_start(eidx_sb[:], eidx[:])
            nc.sync.dma_start(emask_sb[:], emask[:])

            ef_sb = efpool.tile([128, BS * LB], BF16)
            for k in range(NBLK):
                nc.sync.dma_start(ef_sb[32 * k:32 * (k + 1), :],
                                  aef[32 * k:32 * (k + 1), :])

            # ---------- gold emissions (GPSIMD only) ----------
            for k in range(NBLK):
                st = stage.tile([128, BS * T], BF16, name=f"st{k}", tag="st")
                nc.sync.dma_start(st[:], ast[k])
                gath = goldp.tile([128, 2048], BF16, name=f"gath{k}", tag="gath")
                # ISA limit: indirect_copy dst <= 1024 elems/partition
                for hh in range(2):
                    nc.gpsimd.indirect_copy(
                        gath[:, hh * 1024:(hh + 1) * 1024],
                        st[:],
                        eidx_sb[:, k * 128 + hh * 64:k * 128 + (hh + 1) * 64],
                        True,
                    )
                gm = goldp.tile([128, 2048], BF16, name=f"gm{k}", tag="gm")
                nc.gpsimd.tensor_tensor(
                    gm[:], gath[:], emask_sb[:, k * 2048:(k + 1) * 2048], MULT
                )
                par = goldp.tile([128, 2048], F32, name=f"par{k}", tag="par")
                nc.gpsimd.partition_all_reduce(
                    par[:], gm[:], channels=128, reduce_op=bass_isa.ReduceOp.add
                )
                nc.sync.dma_start(emito[k:k + 1, :], par[0:1, :])

            # ---------- recurrence ----------
            efv = ef_sb.rearrange("p (b l) -> p b l", l=LB)

            def ef_ap(t, h):
                k, l_in = divmod(t, LB)
                return efv[32 * k:32 * (k + 1), h * CH:(h + 1) * CH, l_in]

            prev = [None, None]
            stagger_from = None
            for k in range(NBLK):
                ring = ringp.tile([32, LB * BS], BF16, name=f"ring{k}", tag="ring")
                for t_in in range(LB):
                    t = k * LB + t_in
                    for h in range(2):
                        out_ap = ring[:, t_in * BS + h * CH:t_in * BS + (h + 1) * CH]
                        if t == 0:
                            ini = nc.vector.tensor_scalar(
                                out_ap, ef_ap(0, h), estart_sb[:, 0:1], None, MULT
                            )
                            if h == 1 and stagger_from is not None:
                                add_dep_helper(stagger_from.ins, ini.ins,
                                               sync=True, reason="phase stagger")
                        else:
                            u = upool.tile([32, CH], mybir.dt.float32,
                                           name=f"u{h}", tag=f"u{h}")
                            mm = nc.tensor.matmul(
                                u[:], m32_sb[:], prev[h], start=True, stop=True
                            )
                            if t == 1 and h == 0:
                                stagger_from = mm
                            nc.vector.tensor_tensor(out_ap, u[:], ef_ap(t, h), MULT)
                        prev[h] = out_ap
                nc.sync.dma_start(hist[k], ring[:])

    nc.compile()
    return nc


def _host_schedule(feats, transitions):
    """Per-step normalizer schedule C[l] from a 32-sample fp64 sub-simulation."""
    idx = np.linspace(0, feats.shape[0] - 1, 32).astype(np.int64)
    f = feats[idx].astype(np.float64)  # (32, L, T)
    tr = transitions.astype(np.float64)
    C = np.empty(L, np.float64)
    alpha = tr[START][None, :] + f[:, 0]
    C[0] = alpha.max(1).mean()
    eM = np.exp(tr)
    for l in range(1, L):
        m = alpha.max(1, keepdims=True)
        alpha = m + np.log(np.exp(alpha - m) @ eM) + f[:, l]
        C[l] = alpha.max(1).mean()
    return C


def _run(nc, in_maps):
    global LAST_EXEC_NS
    import os
    if os.environ.get("KERNEL_SIM"):
        from types import SimpleNamespace
        from concourse.bass_interp import CoreSim
        outs = []
        ncores = int(os.environ.get("KERNEL_SIM_CORES", str(NCORES)))
        for im in in_maps[:ncores]:
            sim = CoreSim(nc, require_finite=False, require_nnan=False)
            for k, v in im.items():
                sim.tensor(k)[:] = v
            sim.simulate()
            outs.append({n: np.array(sim.tensor(n)) for n in ("hist", "emito")})
        return SimpleNamespace(results=outs, exec_time_ns=None)
    from concourse.bass_utils import run_bass_kernel_spmd
    res = run_bass_kernel_spmd(nc, in_maps, list(range(NCORES)), trace=TRACE)
    LAST_EXEC_NS = res.exec_time_ns
    return res


def kernel(feats, transitions, tags, word_seq_lens):
    global _PROG

    feats = np.asarray(feats, np.float32)
    transitions = np.asarray(transitions, np.float32)
    tags = np.asarray(tags)
    lens = np.asarray(word_seq_lens).astype(np.int64)

    if _PROG is None:
        _PROG = _build_program()
    nc = _PROG

    # ---------------- host-side prep ----------------
    C = _host_schedule(feats, transitions)
    dC = np.diff(C, prepend=0.0)

    trf = transitions.astype(np.float64)
    m32 = np.exp(trf).astype(ml_dtypes.bfloat16)
    estart = np.ascontiguousarray(np.exp(trf[START]).astype(np.float32)[:, None])

    tags64 = tags.astype(np.int64)
    base_mask = (np.arange(L)[None, :] == 0) | (tags64 != 0)  # (B, L)

    in_maps = []
    for core in range(NCORES):
        sl = slice(core * BS, (core + 1) * BS)
        x = feats[sl]                                 # (BS, L, T)
        ex = np.exp(x - dC[None, :, None].astype(np.float32))
        # aef[32k+j, b*128+l_in] = ex[b, 128k+l_in, j]
        aef = np.ascontiguousarray(
            ex.reshape(BS, NBLK, LB, T).transpose(1, 3, 0, 2)
            .reshape(128, BS * LB).astype(ml_dtypes.bfloat16)
        )
        # ast[k, l_in, b*32+j] = x[b, 128k+l_in, j]
        ast = np.ascontiguousarray(
            x.reshape(BS, NBLK, LB, T).transpose(1, 2, 0, 3)
            .reshape(NBLK, LB, BS * T).astype(ml_dtypes.bfloat16)
        )
        tsh = tags64[sl]
        msh = base_mask[sl]
        eidx = np.empty((128, NBLK * 128), np.uint16)
        emask = np.zeros((128, NBLK, BS, 16), np.float32)
        for k in range(NBLK):
            lg = k * LB + np.arange(LB)
            eidx[:, k * 128:(k + 1) * 128] = (
                np.arange(BS)[None, :] * T + tsh[:, lg].T
            ).astype(np.uint16)
            emask[np.arange(LB), k, :, np.arange(LB) % 16] = \
                msh[:, lg].astype(np.float32).T
        emask = emask.reshape(128, NBLK * 2048).astype(ml_dtypes.bfloat16)
        in_maps.append({
            "aef": aef,
            "ast": ast,
            "m32": m32,
            "estart": estart,
            "eidx": eidx,
            "emask": np.ascontiguousarray(emask),
        })

    res = _run(nc, in_maps)
    results = res.results
    ncores_avail = len(results)

    # ---------------- host-side readout (fp64) ----------------
    estop = np.exp(trf[:, STOP])  # (T,)
    total_fwd = 0.0
    total_emit = 0.0
    for core in range(ncores_avail):
        r = results[core]
        h = np.asarray(r["hist"]).astype(np.float64)   # (NBLK, 32, LB*BS)
        em = np.asarray(r["emito"]).astype(np.float64)  # (NBLK, 2048)
        lsh = lens[core * BS:(core + 1) * BS]
        tstar = lsh - 1
        kk, tt = np.divmod(tstar, LB)
        pv = h[kk, :, tt * BS + np.arange(BS)]          # (BS, 32)
        total_fwd += (np.log(pv @ estop) + C[tstar]).sum()
        total_emit += em.reshape(NBLK, BS, 16).sum(axis=(0, 2)).sum()

    # gold transition terms on host
    tg = tags64
    mid_mask = (tg[:, 1:] != 0)
    trans_mid = (trf[tg[:, :-1], tg[:, 1:]] * mid_mask).sum()
    begin = trf[START, tg[:, 0]].sum()
    end_tag = np.take_along_axis(tg, (lens - 1)[:, None], axis=1)[:, 0]
    end = trf[end_tag, STOP].sum()
    total_gold = total_emit + trans_mid + begin + end

    return np.asarray(total_fwd - total_gold, np.float32)



# revision 2
# speedup vs baseline: 2.2657x; 2.2657x over previous
"""BiLSTM-CRF NLL loss kernel for 8 Trainium2 NeuronCores.

Data-parallel over batch (128 samples/core). The partition function runs
as a linear-domain recurrence
    p_t = (M^T p_{t-1}) * exp(feats_t - dc_t)
with a host-computed per-step scalar normalizer schedule dc_t.

Device layout: partitions = 4 sample-groups x 32 tags (128), columns =
32 samples per group. The PE stationary is the 128x128 block-diagonal
diag(eM, eM, eM, eM) (eM = exp(transitions)), loaded ONCE; every
subsequent matmul sets ldweights=False so the PE array weights are
reused, making each step's matmul a 16-column moving pass. Samples are
split into two phase-staggered chains (16 columns each) so the PE->DVE
round-trip latency of one chain hides under the other's work. The DVE
does one [128 x 16] PSUM*SBUF multiply per chain-step, writing bf16
history straight into a ring that streams to DRAM per 128-step block.

Host does everything input-determined in fp64: the normalizer schedule,
the length-indexed readout log(sum_j e^{trans[j,STOP]} p_t*[j]) + C_t,
and the entire gold score (emissions + transitions).
"""
import numpy as np
import ml_dtypes

B, L, T = 1024, 512, 32
START, STOP = 30, 31
NCORES = 8
BS = B // NCORES          # 128 samples per core
NG = 4                    # sample groups stacked on partitions
GS = BS // NG             # 32 samples per group
NBLK = 4                  # l-blocks
LB = L // NBLK            # 128 timesteps per block
CH = GS // 2              # 16 sample-columns per chain

_PROG = None

TRACE = False
LAST_EXEC_NS = None


def _build_program():
    import concourse.bacc as bacc
    import concourse.mybir as mybir
    import concourse.tile as tile
    from concourse.tile_rust import add_dep_helper

    F32 = mybir.dt.float32
    BF16 = mybir.dt.bfloat16
    MULT = mybir.AluOpType.mult

    nc = bacc.Bacc("TRN2", target_bir_lowering=False, debug=False)

    # aef[32g+j, t*32+s] = exp(feats[32g+s, t, j] - dc[t]) bf16
    aef = nc.dram_tensor("aef", [128, L * GS], BF16, kind="ExternalInput").ap()
    m32 = nc.dram_tensor("m32", [128, 128], BF16, kind="ExternalInput").ap()
    estart = nc.dram_tensor("estart", [128, 1], F32, kind="ExternalInput").ap()
    # hist[k, 32g+j, t_in*32+s] = p_t[(g,s), j] at t = 128k + t_in
    hist = nc.dram_tensor("hist", [NBLK, 128, LB * GS], BF16,
                          kind="ExternalOutput").ap()

    with tile.TileContext(nc) as tc:
        with (
            tc.tile_pool(name="consts", bufs=1) as consts,
            tc.tile_pool(name="efpool", bufs=1) as efpool,
            tc.tile_pool(name="ringp", bufs=2) as ringp,
            tc.tile_pool(name="upool", bufs=2, space="PSUM") as upool,
        ):
            m32_sb = consts.tile([128, 128], BF16)
            estart_sb = consts.tile([128, 1], F32)
            nc.sync.dma_start(m32_sb[:], m32[:])
            nc.sync.dma_start(estart_sb[:], estart[:])

            ef_sb = efpool.tile([128, L * GS], BF16)
            for k in range(NBLK):
                nc.sync.dma_start(ef_sb[:, k * LB * GS:(k + 1) * LB * GS],
                                  aef[:, k * LB * GS:(k + 1) * LB * GS])

            def ef_ap(t, h):
                return ef_sb[:, t * GS + h * CH:t * GS + (h + 1) * CH]

            prev = [None, None]
            stagger_from = None
            first_mm = True
            for k in range(NBLK):
                ring = ringp.tile([128, LB * GS], BF16, name=f"ring{k}",
                                  tag="ring")
                for t_in in range(LB):
                    t = k * LB + t_in
                    for h in range(2):
                        out_ap = ring[:, t_in * GS + h * CH:
                                      t_in * GS + (h + 1) * CH]
                        if t == 0:
                            ini = nc.vector.tensor_scalar(
                                out_ap, ef_ap(0, h), estart_sb[:, 0:1],
                                None, MULT
                            )
                            if h == 1 and stagger_from is not None:
                                add_dep_helper(stagger_from.ins, ini.ins,
                                               sync=True,
                                               reason="phase stagger")
                        else:
                            u = upool.tile([128, CH], F32,
                                           name=f"u{h}", tag=f"u{h}")
                            mm = nc.tensor.matmul(
                                u[:], m32_sb[:], prev[h],
                                start=True, stop=True
                            )
                            if first_mm:
                                first_mm = False
                            else:
                                mm.ins.ldweights = False
                            if t == 1 and h == 0:
                                stagger_from = mm
                            nc.vector.tensor_tensor(out_ap, u[:], ef_ap(t, h),
                                                    MULT)
                        prev[h] = out_ap
                nc.sync.dma_start(hist[k], ring[:])

    nc.compile()
    return nc


def _host_schedule(feats, transitions):
    """Per-step normalizer schedule C[l] from a 32-sample fp64 sub-simulation."""
    idx = np.linspace(0, feats.shape[0] - 1, 32).astype(np.int64)
    f = feats[idx].astype(np.float64)  # (32, L, T)
    tr = transitions.astype(np.float64)
    C = np.empty(L, np.float64)
    alpha = tr[START][None, :] + f[:, 0]
    C[0] = alpha.max(1).mean()
    eM = np.exp(tr)
    for l in range(1, L):
        m = alpha.max(1, keepdims=True)
        alpha = m + np.log(np.exp(alpha - m) @ eM) + f[:, l]
        C[l] = alpha.max(1).mean()
    return C


def _run(nc, in_maps):
    global LAST_EXEC_NS
    import os
    if os.environ.get("KERNEL_SIM"):
        from types import SimpleNamespace
        from concourse.bass_interp import CoreSim
        outs = []
        ncores = int(os.environ.get("KERNEL_SIM_CORES", str(NCORES)))
        for im in in_maps[:ncores]:
            sim = CoreSim(nc, require_finite=False, require_nnan=False)
            for k, v in im.items():
                sim.tensor(k)[:] = v
            sim.simulate()
            outs.append({n: np.array(sim.tensor(n)) for n in ("hist",)})
        return SimpleNamespace(results=outs, exec_time_ns=None)
    from concourse.bass_utils import run_bass_kernel_spmd
    res = run_bass_kernel_spmd(nc, in_maps, list(range(NCORES)), trace=TRACE)
    LAST_EXEC_NS = res.exec_time_ns
    return res


def kernel(feats, transitions, tags, word_seq_lens):
    global _PROG

    feats = np.asarray(feats, np.float32)
    transitions = np.asarray(transitions, np.float32)
    tags = np.asarray(tags)
    lens = np.asarray(word_seq_lens).astype(np.int64)

    if _PROG is None:
        _PROG = _build_program()
    nc = _PROG

    # ---------------- host-side prep ----------------
    C = _host_schedule(feats, transitions)
    dC = np.diff(C, prepend=0.0)

    trf = transitions.astype(np.float64)
    eM = np.exp(trf)
    m32 = np.zeros((128, 128), np.float64)
    for g in range(NG):
        m32[32 * g:32 * g + 32, 32 * g:32 * g + 32] = eM
    m32 = m32.astype(ml_dtypes.bfloat16)
    estart = np.ascontiguousarray(
        np.tile(np.exp(trf[START]), NG).astype(np.float32)[:, None])

    in_maps = []
    for core in range(NCORES):
        sl = slice(core * BS, (core + 1) * BS)
        x = feats[sl]                                 # (BS, L, T)
        ex = np.exp(x - dC[None, :, None].astype(np.float32))
        # aef[32g+j, t*32+s] = ex[32g+s, t, j]
        aef = np.ascontiguousarray(
            ex.reshape(NG, GS, L, T).transpose(0, 3, 2, 1)
            .reshape(128, L * GS).astype(ml_dtypes.bfloat16)
        )
        in_maps.append({"aef": aef, "m32": m32, "estart": estart})

    res = _run(nc, in_maps)
    results = res.results
    ncores_avail = len(results)

    # ---------------- host-side readout (fp64) ----------------
    estop = np.exp(trf[:, STOP])  # (T,)
    b_loc = np.arange(BS)
    g_arr = b_loc // GS
    s_arr = b_loc % GS
    total_fwd = 0.0
    for core in range(ncores_avail):
        h = np.asarray(results[core]["hist"]).astype(np.float64)
        # (NBLK, 128, LB*GS) -> [k, g, j, t_in, s]
        h5 = h.reshape(NBLK, NG, 32, LB, GS)
        lsh = lens[core * BS:(core + 1) * BS]
        tstar = lsh - 1
        kk, tt = np.divmod(tstar, LB)
        pv = h5[kk, g_arr, :, tt, s_arr]               # (BS, 32)
        total_fwd += (np.log(pv @ estop) + C[tstar]).sum()

    # ---------------- gold score fully on host (fp64) ----------------
    tg = tags.astype(np.int64)
    emit = np.take_along_axis(feats, tg[:, :, None].astype(np.int64),
                              axis=2)[:, :, 0].astype(np.float64)
    emask = (np.arange(L)[None, :] == 0) | (tg != 0)
    total_emit = (emit * emask).sum()
    mid_mask = (tg[:, 1:] != 0)
    trans_mid = (trf[tg[:, :-1], tg[:, 1:]] * mid_mask).sum()
    begin = trf[START, tg[:, 0]].sum()
    end_tag = np.take_along_axis(tg, (lens - 1)[:, None], axis=1)[:, 0]
    end = trf[end_tag, STOP].sum()
    total_gold = total_emit + trans_mid + begin + end

    return np.asarray(total_fwd - total_gold, np.float32)
